# revision 12
# baseline (speedup 1.0000x reference)
"""Trainium2 Bass kernel for CascadedNN (dense_mlp).

Math (per batch row x of dim 256):
  f  = relu(x @ W1 + b1)           # 512
  f  = relu(f @ W2 + b2)           # 256
  first = sigmoid(f @ Wf + bf)
  a_t = f @ Wc[t,:256] + bc[t]     (t = 0..62)
  p_0 = first;  p_{t+1} = max(sigmoid(a_t + w_t * p_t), p_t),  w_t = Wc[t,256]
  out = [p_0, ..., p_63]           # [B, 64]

Strategy: pure data parallel over 8 cores (8192 rows each), fp16 GEMMs
with fp32 PSUM accumulation, feature-major L1/L2 (x pre-transposed on
the host). The head runs batch-major - each [128 feat, 128 batch] f2
block is the stationary operand against Wcat [256, 64], landing
[128 batch, 64 steps] tiles in PSUM with t along the free dim.

The 63-step cascade is run as two fixed-point passes (contraction
factor q = 0.25*max|w| ~ 0.04, error <= 0.5*q^2 < 1e-3):
  pass A: p~_t = cummax_{s<=t} sigmoid(ZA_s), ZA = head logits with
          bc + 0.5*w + C folded in (prev ~= 0.5 linearization).
  pass B: s_t = sigmoid(ZA_t + w_t*(p~_{t-1}-0.5)); out = cummax(s_t)

Key tricks vs a naive lowering:
  - bias (bc + 0.5w + C) folded into the head GEMM as a K=2 rank-1
    matmul with fp16 hi/lo rows: PSUM holds finished pass-A logits.
  - sigmoid is monotone, so cummax(sigmoid(z)) = sigmoid(cummax(z)):
    pass A scans the raw PSUM logits (fp32!) and sigmoids the scan.
  - the +C (=16) offset keeps all logits positive, which lets ONE
    [128,512] tensor_tensor_scan with op0=mult on a 0/1 column mask
    handle all eight 64-step groups (state resets to max(0, z) at each
    group boundary).
  - evacuations are [128,1024] double-bank ops split between ACT and
    DVE (gpsimd has no PSUM port); gpsimd runs the SBUF-only head ops
    (scalar_tensor_tensor, copies, pass-B scan) and the output DMA.
  - x loads are hoisted upfront on the sync HWDGE ring; output stores
    ride the gpsimd SWDGE ring, so stores never head-of-line block
    loads.
"""

import numpy as np
import ml_dtypes
from contextlib import ExitStack

import concourse.bacc as bacc
import concourse.bass as bass
import concourse.mybir as mybir
from concourse import tile
from concourse.bass_utils import run_bass_kernel_spmd

# Both Relu (L1/L2 evac) and Sigmoid (cascade) live in the
# "sigmoid_and_others" activation table. Left alone, walrus assigns Relu
# to the first table containing it ("exp_and_others") and Sigmoid to
# this one, forcing two 1.3us table reloads per loop iteration on the
# ACT engine. Empty out every other table so all activations resolve to
# the shared one (dict order, hence act_func_set_id, is preserved).
_ORIG_GAT = bacc.get_activation_tables


def _gat_one_table(arch):
    tabs = _ORIG_GAT(arch)
    return {name: (funcs if name == "sigmoid_and_others" else set())
            for name, funcs in tabs.items()}


bacc.get_activation_tables = _gat_one_table

FP16 = mybir.dt.float16
F32 = mybir.dt.float32
AF = mybir.ActivationFunctionType
OP = mybir.AluOpType

B, D, H1, H2, T = 65536, 256, 512, 256, 64
NCORES = 8
BL = B // NCORES            # 8192 rows per core
NCHUNK = 4
CB = BL // NCHUNK           # 2048 rows per chunk
NPAIR = CB // 1024          # [128,1024] psum pairs per chunk row-block
NBANK = 2                   # head psum banks per chunk (each 8 j-tiles)
COFF = 16.0                 # logit offset keeping head logits positive

_CACHE = {}


def _build(bench_nrep=0, rev="r1", evac_pat="AADADAADADAA", xup=True):
    """evac_pat: engine rotation for L1/L2 psum pair evacuation
    (A=ACT, D=DVE), consumed round-robin, 12 pairs per chunk."""
    nc = bacc.Bacc("TRN2", target_bir_lowering=False, debug=False,
                   num_devices=NCORES)
    # unique per-variant dummy input: defeats NEFF/executable cache
    # collisions between structurally-different builds with identical I/O
    vtag = nc.dram_tensor(
        f"vtag_r{bench_nrep}e{evac_pat}x{int(xup)}v{rev}",
        [1, 1], F32, kind="ExternalInput")

    xt = nc.dram_tensor("xt", [2, 128, BL], FP16, kind="ExternalInput")
    w1 = nc.dram_tensor("w1", [2, 128, H1], FP16, kind="ExternalInput")
    b1 = nc.dram_tensor("b1", [4, 128, 1], F32, kind="ExternalInput")
    w2 = nc.dram_tensor("w2", [4, 128, H2], FP16, kind="ExternalInput")
    b2 = nc.dram_tensor("b2", [2, 128, 1], F32, kind="ExternalInput")
    wcat = nc.dram_tensor("wcat", [2, 128, T], FP16, kind="ExternalInput")
    brow = nc.dram_tensor("brow", [2, 512], FP16, kind="ExternalInput")
    ones2 = nc.dram_tensor("ones2", [2, 128], FP16, kind="ExternalInput")
    wgt = nc.dram_tensor("wgt", [128, 512], FP16, kind="ExternalInput")
    maskt = nc.dram_tensor("maskt", [128, 512], FP16, kind="ExternalInput")
    negc = nc.dram_tensor("negc", [128, 1], F32, kind="ExternalInput")
    negch = nc.dram_tensor("negch", [128, 1], F32, kind="ExternalInput")
    out = nc.dram_tensor("out", [BL, T], FP16, kind="ExternalOutput")

    with tile.TileContext(nc) as tc, ExitStack() as ctx:
        wpool = ctx.enter_context(tc.tile_pool(name="wts", bufs=1))
        xpool = ctx.enter_context(tc.tile_pool(name="xin", bufs=1))
        f1pool = ctx.enter_context(tc.tile_pool(name="f1", bufs=2))
        f2pool = ctx.enter_context(tc.tile_pool(name="f2", bufs=2))
        hpool = ctx.enter_context(tc.tile_pool(name="hd", bufs=3))
        pspool = ctx.enter_context(
            tc.tile_pool(name="ps", bufs=3, space=bass.MemorySpace.PSUM))

        # resident weights / constants
        w1sb = [wpool.tile([128, H1], FP16, name=f"w1_{k}", tag=f"w1_{k}")
                for k in range(2)]
        w2sb = [wpool.tile([128, H2], FP16, name=f"w2_{k}", tag=f"w2_{k}")
                for k in range(4)]
        wcsb = [wpool.tile([128, T], FP16, name=f"wc_{k}", tag=f"wc_{k}")
                for k in range(2)]
        b1sb = [wpool.tile([128, 1], F32, name=f"b1_{m}", tag=f"b1_{m}")
                for m in range(4)]
        b2sb = [wpool.tile([128, 1], F32, name=f"b2_{m}", tag=f"b2_{m}")
                for m in range(2)]
        brsb = wpool.tile([2, 512], FP16, name="br", tag="br")
        onsb = wpool.tile([2, 128], FP16, name="on", tag="on")
        wgsb = wpool.tile([128, 512], FP16, name="wg", tag="wg")
        masksb = wpool.tile([128, 512], FP16, name="mk", tag="mk")
        ncsb = wpool.tile([128, 1], F32, name="ncf", tag="ncf")
        nchsb = wpool.tile([128, 1], F32, name="nch", tag="nch")
        vtsb = wpool.tile([1, 1], F32, name="vt", tag="vt")

        # w1 first on the sync ring (needed by the very first matmul);
        # everything else resident on the gpsimd SWDGE ring.
        for k in range(2):
            nc.sync.dma_start(w1sb[k][:], w1[k])
        nc.gpsimd.dma_start(vtsb[:], vtag[:])
        for k in range(4):
            nc.gpsimd.dma_start(w2sb[k][:], w2[k])
            nc.gpsimd.dma_start(b1sb[k][:], b1[k])
        for k in range(2):
            nc.gpsimd.dma_start(wcsb[k][:], wcat[k])
            nc.gpsimd.dma_start(b2sb[k][:], b2[k])
        nc.gpsimd.dma_start(brsb[:], brow[:])
        nc.gpsimd.dma_start(onsb[:], ones2[:])
        nc.gpsimd.dma_start(wgsb[:], wgt[:])
        nc.gpsimd.dma_start(masksb[:], maskt[:])
        nc.gpsimd.dma_start(ncsb[:], negc[:])
        nc.gpsimd.dma_start(nchsb[:], negch[:])

        wg3 = wgsb[:].rearrange("p (g t) -> p g t", t=T)

        # pre-loop dummy activation: puts the (single) act table load on
        # the loop-preheader path so the fixpoint pass hoists it out of
        # the For_i body.
        dummy = wpool.tile([1, 1], F32, name="du", tag="du")
        nc.scalar.activation(dummy[:], vtsb[:], AF.Sigmoid)

        # output view: out[f*128 + p, t] <- OUT[p, f_within, t]
        ov = out[:].rearrange("(f p) t -> p f t", p=128)

        loop = tc.For_i(0, bench_nrep, 1) if bench_nrep else None
        if loop is not None:
            loop.__enter__()

        # all x loads up front on the (otherwise idle) sync HWDGE ring:
        # loads never queue behind output stores, and chunk c+1's data is
        # in flight while chunk c computes.
        xsb = [[xpool.tile([128, CB], FP16, name=f"x{c}_{k}",
                           tag=f"x{c}_{k}") for k in range(2)]
               for c in range(NCHUNK)]
        if xup:
            for c in range(NCHUNK):
                for k in range(2):
                    nc.sync.dma_start(xsb[c][k][:], xt[k][:, bass.ts(c, CB)])

        ev = [0]

        def evac_relu(out_ap, in_ap, bias_ap):
            e = evac_pat[ev[0] % len(evac_pat)]
            ev[0] += 1
            if e == "A":
                nc.scalar.activation(out_ap, in_ap, AF.Relu, bias=bias_ap,
                                     scale=1.0)
            else:
                nc.vector.tensor_scalar(out_ap, in_ap, bias_ap, 0.0,
                                        OP.add, OP.max)

        for c in range(NCHUNK):
            if not xup:
                for k in range(2):
                    nc.sync.dma_start(xsb[c][k][:], xt[k][:, bass.ts(c, CB)])

            def layer(nk, wsb, insb, outsb, bsb):
                # per m-tile: NPAIR [128,1024] psum pairs, k-outer for
                # stationary-weight reuse across the 4 nb quarters.
                for m in range(len(outsb)):
                    prs = [pspool.tile([128, 1024], F32, name="ps",
                                       tag="ps", bufs=3)
                           for _ in range(NPAIR)]
                    for k in range(nk):
                        for nb in range(2 * NPAIR):
                            nc.tensor.matmul(
                                prs[nb // 2][:, bass.ts(nb % 2, 512)],
                                wsb[k][:, bass.ts(m, 128)],
                                insb[k][:, bass.ts(nb, 512)],
                                start=(k == 0), stop=(k == nk - 1))
                    for pr in range(NPAIR):
                        evac_relu(outsb[m][:, bass.ts(pr, 1024)],
                                  prs[pr][:], bsb[m][:])

            # L1: f1[m] = relu(W1.T @ x + b1), feature-major fp16
            f1sb = [f1pool.tile([128, CB], FP16, name=f"f1_{m}",
                                tag=f"f1_{m}") for m in range(4)]
            layer(2, w1sb, xsb[c], f1sb, b1sb)

            # L2: f2[m] = relu(W2.T @ f1 + b2)
            f2sb = [f2pool.tile([128, CB], FP16, name=f"f2_{m}",
                                tag=f"f2_{m}") for m in range(2)]
            layer(4, w2sb, f1sb, f2sb, b2sb)

            # head, batch-major: bias via K=2 rank-1 matmul (hi/lo fp16
            # rows add bc + 0.5w + C exactly), then per 128-row tile j,
            # f2_tile.T @ Wcat -> [128 batch, 64 steps]; 8 tiles per bank.
            banks = []
            for bi in range(NBANK):
                psw = pspool.tile([128, 512], F32, name="psw", tag="psh",
                                  bufs=2)
                nc.tensor.matmul(psw[:], onsb[:], brsb[:],
                                 start=True, stop=False,
                                 skip_group_check=True)
                for j8 in range(8):
                    j = bi * 8 + j8
                    for k in range(2):
                        nc.tensor.matmul(
                            psw[:, bass.ts(j8, T)],
                            f2sb[k][:, bass.ts(j, 128)], wcsb[k][:],
                            start=False, stop=(k == 1),
                            skip_group_check=True)
                banks.append(psw)

            # head post-processing, stage-by-stage across both banks;
            # engines rotate DVE -> ACT -> Pool -> DVE -> ACT -> DVE.
            # (Pool supports only TensorTensor/TensorCopy/DMA: the
            # "p~ - 0.5" shift rides the tanh identity
            # sigmoid(z) - 0.5 = tanh(z/2)/2, so Pool's op is a plain
            # multiply by the pre-halved w row.)
            Ms, TMs, TMPs, ZBs, SBs = [], [], [], [], []
            for bi in range(NBANK):
                # pass A: masked cummax of raw logits straight from PSUM
                M = hpool.tile([128, 512], F32, name=f"m_{bi}",
                               tag=f"m_{bi}")
                nc.vector.tensor_tensor_scan(M[:], masksb[:], banks[bi][:],
                                             0.0, OP.mult, OP.max)
                Ms.append(M)
            for bi in range(NBANK):
                # TM = tanh((M-C)/2) = 2*sigmoid(M-C) - 1 = 2*p~ - 1
                TM = hpool.tile([128, 512], FP16, name=f"sm_{bi}",
                                tag=f"sm_{bi}")
                nc.scalar.activation(TM[:], Ms[bi][:], AF.Tanh,
                                     bias=nchsb[:], scale=0.5)
                TMs.append(TM)
            for bi in range(NBANK):
                # w_t * (p~_{t-1} - 0.5) = (w_t/2) * TM_{t-1}
                TMP = hpool.tile([128, 512], FP16, name=f"tp_{bi}",
                                 tag=f"tp_{bi}")
                t3 = TMP[:].rearrange("p (g t) -> p g t", t=T)
                s3 = TMs[bi][:].rearrange("p (g t) -> p g t", t=T)
                nc.gpsimd.tensor_tensor(t3[:, :, 1:], s3[:, :, 0:T - 1],
                                        wg3[:, :, 1:], OP.mult)
                TMPs.append(TMP)
            for bi in range(NBANK):
                # pass-B logits: zB = ZA + w*(p~-0.5), PSUM read -> DVE
                ZB = hpool.tile([128, 512], F32, name=f"zb_{bi}",
                                tag=f"zb_{bi}")
                z3 = ZB[:].rearrange("p (g t) -> p g t", t=T)
                t3 = TMPs[bi][:].rearrange("p (g t) -> p g t", t=T)
                p3 = banks[bi][:].rearrange("p (g t) -> p g t", t=T)
                nc.vector.tensor_tensor(z3[:, :, 1:], t3[:, :, 1:],
                                        p3[:, :, 1:], OP.add)
                ZBs.append(ZB)
            for bi in range(NBANK):
                SB = hpool.tile([128, 512], FP16, name=f"sb_{bi}",
                                tag=f"sb_{bi}")
                s3 = SB[:].rearrange("p (g t) -> p g t", t=T)
                z3 = ZBs[bi][:].rearrange("p (g t) -> p g t", t=T)
                tm3 = TMs[bi][:].rearrange("p (g t) -> p g t", t=T)
                nc.scalar.activation(s3[:, :, 1:], z3[:, :, 1:], AF.Sigmoid,
                                     bias=ncsb[:], scale=1.0)
                # col 0 = sigmoid(M_0 - C) = 0.5*TM_0 + 0.5
                nc.scalar.activation(s3[:, :, 0:1], tm3[:, :, 0:1],
                                     AF.Copy, bias=0.5, scale=0.5)
                SBs.append(SB)
            for bi in range(NBANK):
                OUTt = hpool.tile([128, 512], FP16, name=f"ou_{bi}",
                                  tag=f"ou_{bi}")
                # scan is a DVE-only instruction (walrus rejects it on Pool)
                nc.vector.tensor_tensor_scan(OUTt[:], masksb[:], SBs[bi][:],
                                             0.0, OP.mult, OP.max)
                o3 = OUTt[:].rearrange("p (g t) -> p g t", t=T)
                fbase = c * (CB // 128) + bi * 8
                nc.gpsimd.dma_start(ov[:, fbase:fbase + 8, :], o3[:, :, :])

        if loop is not None:
            loop.__exit__(None, None, None)

    nc.compile()
    return nc


def _prep_shared(W1, b1, W2, b2, Wf, bf, Wc, bc):
    fp16 = np.float16
    f32 = np.float32
    W1 = np.asarray(W1, f32)
    W2 = np.asarray(W2, f32)
    Wf = np.asarray(Wf, f32)
    Wc = np.asarray(Wc, f32)
    d = {}
    d["w1"] = np.ascontiguousarray(W1.astype(fp16).reshape(2, 128, H1))
    d["w2"] = np.ascontiguousarray(W2.astype(fp16).reshape(4, 128, H2))
    wcat = np.concatenate([Wf, Wc[:, :H2].T], axis=1)   # [256, 64]
    d["wcat"] = np.ascontiguousarray(wcat.astype(fp16).reshape(2, 128, T))
    d["b1"] = np.ascontiguousarray(np.asarray(b1, f32).reshape(4, 128, 1))
    d["b2"] = np.ascontiguousarray(np.asarray(b2, f32).reshape(2, 128, 1))
    bcat = np.concatenate([np.asarray(bf, f32), np.asarray(bc, f32)])
    wprev = Wc[:, H2]                                   # [63]
    wrow = np.concatenate([np.zeros(1, f32), wprev])    # [64], 0 at t=0
    # pass A uses constant prev=0.5: fold 0.5*w_t (plus the positivity
    # offset C) into the head bias, applied as an exact hi/lo fp16 pair
    brow = np.tile(bcat + 0.5 * wrow + COFF, 8).astype(f32)   # [512]
    bhi = brow.astype(fp16)
    blo = (brow - bhi.astype(f32)).astype(fp16)
    d["brow"] = np.ascontiguousarray(np.stack([bhi, blo]))    # [2, 512]
    d["ones2"] = np.ones((2, 128), fp16)
    # pre-halved: w*(p~-0.5) is computed as (w/2) * tanh((M-C)/2)
    d["wgt"] = np.ascontiguousarray(
        np.tile(0.5 * wrow, (128, 8)).astype(fp16))     # [128, 512]
    mask = np.ones((128, 512), fp16)
    mask[:, 0::T] = 0.0
    d["maskt"] = mask
    d["negc"] = np.full((128, 1), -COFF, f32)
    d["negch"] = np.full((128, 1), -0.5 * COFF, f32)
    return d


def _core_inputs(x, shared, c):
    fp16 = np.float16
    xs = x[c * BL:(c + 1) * BL, :]
    m = dict(shared)
    m["xt"] = np.ascontiguousarray(xs.T.astype(fp16)).reshape(2, 128, BL)
    return m


def kernel(x, W1, b1, W2, b2, Wf, bf, Wc, bc):
    if "nc" not in _CACHE:
        _CACHE["nc"] = _build()
    nc = _CACHE["nc"]

    x = np.asarray(x, np.float32)
    shared = _prep_shared(W1, b1, W2, b2, Wf, bf, Wc, bc)
    in_maps = [_core_inputs(x, shared, c) for c in range(NCORES)]

    # zero-fill any declared inputs we don't feed (e.g. the variant tag)
    pname = nc.partition_id_tensor.name if nc.partition_id_tensor else None
    for alloc in nc.m.functions[0].allocations:
        if (isinstance(alloc, mybir.MemoryLocationSet)
                and alloc.kind == "ExternalInput"):
            nm = alloc.memorylocations[0].name
            if nm != pname:
                for m in in_maps:
                    if nm not in m:
                        m[nm] = np.zeros(tuple(alloc.tensor_shape),
                                         mybir.dt.np(alloc.dtype))

    res = run_bass_kernel_spmd(nc, in_maps, list(range(NCORES)))
    outs = [np.asarray(res.results[c]["out"], np.float32)
            for c in range(NCORES)]
    return np.concatenate(outs, axis=0)
